# revision 19
# baseline (speedup 1.0000x reference)
"""Trainium2 Bass kernel for the BF16Indexer sparse-attention problem.

Computes, for B=1, M=2048, H=32, D=128, N=4096:
    logits = einsum('bmhd,bnd->bmhn', q, k)          (fp32 accum)
    o      = einsum('bmhn,bmh->bmn', relu(logits), w) / sqrt(D)

Sharding: M (query tokens) split across 8 cores; k replicated.

Per-core algorithm (M_loc = 256 rows, mh = M_loc*H = 8192):
  - qT  [128=d, mh]     (host-transposed shard of q)
  - kT  [128=d, N]      (host-transposed k, replicated)
  - wblk[128, n_tiles*128]  block-diagonal per-tile weight matrices
  - mm1 (PE):  for each mh-tile t (128 rows = 4 m's x 32 h):
        p1 = qT[:, t].T @ kT[:, chunk]         -> logits [128, 512] fp32 PSUM
  - drain (ACT/DVE alternating): y = relu(softmax_scale * p1) -> bf16 SBUF
  - mm2 (PE):  p2[:, chunk] += wblk[:, t].T @ y  accumulated over the 32
        tiles of a group (block-diagonal lhsT routes each tile's 4 m's to
        the right 4 of 128 output partitions)
  - p2 [128=m, 2048=n] fp32 -> SBUF -> DMA to o[m, n]

kernel(**inputs) takes the FULL inputs and returns the FULL (1, 2048, 4096)
fp32 output; sharding/gather is host-side marshalling only (no host FLOPs).
"""

import math
import numpy as np
import ml_dtypes

import concourse.bass as bass
import concourse.mybir as mybir
import concourse.tile as tile
from concourse import bacc
from concourse.bass_utils import run_bass_kernel_spmd

# Problem constants (hardcoded per harness contract)
B, M, H, D, N = 1, 2048, 32, 128, 4096
N_CORES = 8
M_LOC = M // N_CORES              # 256 query rows per core
MH = M_LOC * H                    # 8192
N_TILES = MH // 128               # 64 mh-tiles (4 m's each)
SOFTMAX_SCALE = 1.0 / math.sqrt(float(D))


def build_nc(m_loc=M_LOC, n=N, group_tiles=32, n_chunk=1024):
    """Build + compile the per-core bass program.

    group_tiles: mh-tiles per mm2 accumulation group (psum2 has
                 4*group_tiles output partitions).
    n_chunk:     n-columns processed per (group, half) pass; psum2 is
                 [128, n_chunk] fp32 = n_chunk/512 PSUM banks.
    """
    mh = m_loc * H
    n_tiles = mh // 128
    assert n_tiles % group_tiles == 0
    n_groups = n_tiles // group_tiles
    assert n % n_chunk == 0
    n_halves = n // n_chunk
    assert n_chunk % 512 == 0
    c_per_half = n_chunk // 512
    gp = 4 * group_tiles  # output partitions per group

    nc = bacc.Bacc("TRN2", target_bir_lowering=False, debug=False)

    bf16 = mybir.dt.bfloat16
    f32 = mybir.dt.float32

    qT_d = nc.dram_tensor("qT", [128, mh], bf16, kind="ExternalInput")
    kT_d = nc.dram_tensor("kT", [128, n], bf16, kind="ExternalInput")
    wblk_d = nc.dram_tensor("wblk", [128, n_tiles * gp], bf16, kind="ExternalInput")
    o_d = nc.dram_tensor("o", [m_loc, n], f32, kind="ExternalOutput")

    with tile.TileContext(nc) as tc:
        with (
            tc.tile_pool(name="const", bufs=1) as const_pool,
            tc.tile_pool(name="ypool", bufs=5) as ypool,
            tc.tile_pool(name="psum1", bufs=6, space="PSUM") as psum1,
            tc.tile_pool(name="psum2", bufs=2, space="PSUM") as psum2,
            tc.tile_pool(name="ostage", bufs=4) as ostage,
        ):
            qT = const_pool.tile([128, mh], bf16)
            kT = const_pool.tile([128, n], bf16)
            wblk = const_pool.tile([128, n_tiles * gp], bf16)

            # warm the ACT spline tables while DMAs run
            warm = const_pool.tile([128, 1], bf16)
            nc.gpsimd.memset(warm[:], 0)
            nc.scalar.activation(warm[:], warm[:],
                                 mybir.ActivationFunctionType.Relu)

            # warm the PE (HAM un-throttles after ~3.4us of activity) with
            # small matmuls on a zeroed scratch tile while DMAs run
            if n_tiles >= 16:
                wsrc = const_pool.tile([128, 128], bf16)
                nc.gpsimd.memset(wsrc[:], 0)
                wps = psum1.tile([128, 128], f32, tag="p1", name="warm_ps")
                for _ in range(45):
                    nc.tensor.matmul(wps[:], wsrc[:], wsrc[:],
                                     start=True, stop=True)

            # Hand-scheduled demand-ordered loads on the 3 HWDGE queues
            # (~26GB/s each). Pass 1 consumes qT+wblk at ~37GB/s each, so
            # qT is fed by two queues early while scalar streams wblk.
            wb_n = n_tiles * gp
            SY, GP, SC = nc.sync, nc.gpsimd, nc.scalar
            sched = [
                (SY, kT, kT_d, 0, 512),
                (SC, kT, kT_d, 512, 1024),
                (GP, qT, qT_d, 0, 128),
                (GP, qT, qT_d, 128, 1024),
                (SC, wblk, wblk_d, 0, 512),
                (SY, qT, qT_d, 1024, 2048),
                (SC, wblk, wblk_d, 512, 1536),
                (GP, qT, qT_d, 2048, 3072),
                (SC, wblk, wblk_d, 1536, 2560),
                (SY, qT, qT_d, 3072, 4096),
                (GP, wblk, wblk_d, 2560, 3584),
                (SC, wblk, wblk_d, 3584, 4096),
                (SY, kT, kT_d, 1024, 2048),
                (GP, kT, kT_d, 2048, 3072),
                (SC, kT, kT_d, 3072, 4096),
                (SY, qT, qT_d, 4096, 5120),
                (GP, qT, qT_d, 5120, 6144),
                (SC, wblk, wblk_d, 4096, 5120),
                (SY, wblk, wblk_d, 5120, 6144),
                (GP, qT, qT_d, 6144, 7168),
                (SC, wblk, wblk_d, 6144, 7168),
                (SY, qT, qT_d, 7168, 8192),
                (GP, wblk, wblk_d, 7168, 8192),
            ]
            widths = {id(kT): n, id(qT): mh, id(wblk): wb_n}
            seen = set()
            for eng, dst, src, lo, hi in sched:
                hi = min(hi, widths[id(dst)])
                lo = min(lo, hi)
                key = (id(dst), lo, hi)
                if hi <= lo or key in seen:
                    continue
                seen.add(key)
                eng.dma_start(dst[:, lo:hi], src[:, lo:hi])
            # cover any remainder (larger-than-default configs)
            for dst, src, w in ((kT, kT_d, n), (qT, qT_d, mh), (wblk, wblk_d, wb_n)):
                done = max((h for (i_, l_, h) in seen if i_ == id(dst)), default=0)
                if done < w:
                    SY.dma_start(dst[:, done:], src[:, done:])

            def emit_mm1(g, hf, t):
                """mm1 for one mh-tile: c_per_half [128,512] psum tiles, each
                drained (relu+scale -> bf16) on a fixed engine per chunk."""
                tg = g * group_tiles + t
                qT_t = qT[:, bass.ts(tg, 128)]
                y_t = ypool.tile([128, n_chunk], bf16, tag="y")
                for c in range(c_per_half):
                    p1 = psum1.tile([128, 512], f32)
                    nc.tensor.matmul(
                        p1[:],
                        qT_t,
                        kT[:, bass.ds(hf * n_chunk + c * 512, 512)],
                        start=True,
                        stop=True,
                    )
                    ysl = y_t[:, bass.ts(c, 512)]
                    if (t * c_per_half + c) % 2 == 0:
                        nc.scalar.activation(
                            ysl, p1[:],
                            mybir.ActivationFunctionType.Relu,
                            scale=SOFTMAX_SCALE,
                        )
                    else:
                        nc.vector.tensor_scalar(
                            ysl, p1[:], SOFTMAX_SCALE, 0.0,
                            mybir.AluOpType.mult, mybir.AluOpType.max,
                        )
                return y_t

            def emit_mm2(p2_chunks, g, t, y_t):
                tg = g * group_tiles + t
                w_t = wblk[:, bass.ts(tg, gp)]
                for c in range(c_per_half):
                    nc.tensor.matmul(
                        p2_chunks[c][:],
                        w_t,
                        y_t[:, bass.ts(c, 512)],
                        start=(t == 0),
                        stop=(t == group_tiles - 1),
                    )

            DELAY = 2  # tiles of run-ahead before mm2 consumes a drained y
            for g in range(n_groups):
                for hf in range(n_halves):
                    p2_chunks = [psum2.tile([gp, 512], f32, tag="p2", name=f"p2_{g}_{hf}_{c}")
                                 for c in range(c_per_half)]
                    ys = []
                    for t in range(group_tiles):
                        ys.append(emit_mm1(g, hf, t))
                        if t >= DELAY:
                            emit_mm2(p2_chunks, g, t - DELAY, ys[t - DELAY])
                    for t in range(group_tiles - DELAY, group_tiles):
                        emit_mm2(p2_chunks, g, t, ys[t])
                    # per-chunk psum2 drain, alternating engines; stores
                    # split by partition halves across rotating queues so
                    # the final store isn't one long single-queue DMA
                    for c in range(c_per_half):
                        ost = ostage.tile([gp, 512], f32)
                        if (hf * c_per_half + c) % 2 == 0:
                            nc.vector.tensor_copy(ost[:], p2_chunks[c][:])
                        else:
                            nc.scalar.copy(ost[:], p2_chunks[c][:])
                        hp = gp // 2
                        qa = [nc.sync, nc.gpsimd, nc.scalar]
                        base = (g * n_halves + hf) * c_per_half + c
                        for piece in range(2):
                            rows = bass.ds(g * gp + piece * hp, hp)
                            nc.sync if False else None
                            qa[(base + piece) % 3].dma_start(
                                o_d[rows, bass.ds(hf * n_chunk + c * 512, 512)],
                                ost[bass.ds(piece * hp, hp), :],
                            )

    nc.compile()
    return nc


def marshal_core_inputs(q, k, weights, core, m_loc=M_LOC, group_tiles=32):
    """Host-side layout marshalling for one core (no arithmetic)."""
    n_tiles = (m_loc * H) // 128
    gp = 4 * group_tiles
    bf16 = ml_dtypes.bfloat16

    q_sh = np.asarray(q[0, core * m_loc:(core + 1) * m_loc])   # (m_loc, H, D) bf16
    qT = np.ascontiguousarray(q_sh.reshape(m_loc * H, D).T)     # (128, mh)
    kT = np.ascontiguousarray(np.asarray(k[0]).T)               # (128, n)

    w_sh = np.asarray(weights[core * m_loc:(core + 1) * m_loc, 0, :])  # (m_loc, H)
    # wblk[row, tg*gp + col]: for tile tg (4 m's), local m j (0..3), head h:
    #   row = 32*j + h, col = 4*(tg % group_tiles) + j  -> w[m, h]
    wblk = np.zeros((n_tiles, 128, gp), dtype=bf16)
    w_r = w_sh.reshape(n_tiles, 4, H)                           # (tg, j, h)
    tgs = np.arange(n_tiles)
    for j in range(4):
        cols = 4 * (tgs % group_tiles) + j                      # (tg,)
        wblk[tgs[:, None], 32 * j + np.arange(H)[None, :], cols[:, None]] = w_r[:, j, :]
    wblk = np.ascontiguousarray(wblk.transpose(1, 0, 2).reshape(128, n_tiles * gp))

    return {"qT": qT, "kT": kT, "wblk": wblk}


_NC_CACHE = {}


def _get_nc():
    if "nc" not in _NC_CACHE:
        _NC_CACHE["nc"] = build_nc()
    return _NC_CACHE["nc"]


def kernel(q, k, weights):
    nc = _get_nc()
    in_maps = [marshal_core_inputs(q, k, weights, c) for c in range(N_CORES)]
    res = run_bass_kernel_spmd(nc, in_maps, list(range(N_CORES)))
    out = np.concatenate([res.results[c]["o"] for c in range(N_CORES)], axis=0)
    return out[None]  # (1, M, N) fp32


# revision 20
# speedup vs baseline: 1.0036x; 1.0036x over previous
"""Trainium2 Bass kernel for the BF16Indexer sparse-attention problem.

Computes, for B=1, M=2048, H=32, D=128, N=4096:
    logits = einsum('bmhd,bnd->bmhn', q, k)          (fp32 accum)
    o      = einsum('bmhn,bmh->bmn', relu(logits), w) / sqrt(D)

Sharding: M (query tokens) split across 8 cores; k replicated.

Per-core algorithm (M_loc = 256 rows, mh = M_loc*H = 8192):
  - qT  [128=d, mh]     (host-transposed shard of q)
  - kT  [128=d, N]      (host-transposed k, replicated)
  - wblk[128, n_tiles*128]  block-diagonal per-tile weight matrices
  - mm1 (PE):  for each mh-tile t (128 rows = 4 m's x 32 h):
        p1 = qT[:, t].T @ kT[:, chunk]         -> logits [128, 512] fp32 PSUM
  - drain (ACT/DVE alternating): y = relu(softmax_scale * p1) -> bf16 SBUF
  - mm2 (PE):  p2[:, chunk] += wblk[:, t].T @ y  accumulated over the 32
        tiles of a group (block-diagonal lhsT routes each tile's 4 m's to
        the right 4 of 128 output partitions)
  - p2 [128=m, 2048=n] fp32 -> SBUF -> DMA to o[m, n]

kernel(**inputs) takes the FULL inputs and returns the FULL (1, 2048, 4096)
fp32 output; sharding/gather is host-side marshalling only (no host FLOPs).
"""

import math
import numpy as np
import ml_dtypes

import concourse.bass as bass
import concourse.mybir as mybir
import concourse.tile as tile
from concourse import bacc
from concourse.bass_utils import run_bass_kernel_spmd

# Problem constants (hardcoded per harness contract)
B, M, H, D, N = 1, 2048, 32, 128, 4096
N_CORES = 8
M_LOC = M // N_CORES              # 256 query rows per core
MH = M_LOC * H                    # 8192
N_TILES = MH // 128               # 64 mh-tiles (4 m's each)
SOFTMAX_SCALE = 1.0 / math.sqrt(float(D))


def build_nc(m_loc=M_LOC, n=N, group_tiles=32, n_chunk=1024):
    """Build + compile the per-core bass program.

    group_tiles: mh-tiles per mm2 accumulation group (psum2 has
                 4*group_tiles output partitions).
    n_chunk:     n-columns processed per (group, half) pass; psum2 is
                 [128, n_chunk] fp32 = n_chunk/512 PSUM banks.
    """
    mh = m_loc * H
    n_tiles = mh // 128
    assert n_tiles % group_tiles == 0
    n_groups = n_tiles // group_tiles
    assert n % n_chunk == 0
    n_halves = n // n_chunk
    assert n_chunk % 512 == 0
    c_per_half = n_chunk // 512
    gp = 4 * group_tiles  # output partitions per group

    nc = bacc.Bacc("TRN2", target_bir_lowering=False, debug=False)

    bf16 = mybir.dt.bfloat16
    f32 = mybir.dt.float32

    qT_d = nc.dram_tensor("qT", [128, mh], bf16, kind="ExternalInput")
    kT_d = nc.dram_tensor("kT", [128, n], bf16, kind="ExternalInput")
    wblk_d = nc.dram_tensor("wblk", [128, n_tiles * gp], bf16, kind="ExternalInput")
    o_d = nc.dram_tensor("o", [m_loc, n], f32, kind="ExternalOutput")

    with tile.TileContext(nc) as tc:
        with (
            tc.tile_pool(name="const", bufs=1) as const_pool,
            tc.tile_pool(name="ypool", bufs=5) as ypool,
            tc.tile_pool(name="psum1", bufs=6, space="PSUM") as psum1,
            tc.tile_pool(name="psum2", bufs=2, space="PSUM") as psum2,
            tc.tile_pool(name="ostage", bufs=4) as ostage,
        ):
            qT = const_pool.tile([128, mh], bf16)
            kT = const_pool.tile([128, n], bf16)
            wblk = const_pool.tile([128, n_tiles * gp], bf16)

            # Hand-scheduled demand-ordered loads on the 3 HWDGE queues
            # (~26GB/s each). Pass 1 consumes qT+wblk at ~37GB/s each, so
            # qT is fed by two queues early while scalar streams wblk.
            wb_n = n_tiles * gp
            SY, GP, SC = nc.sync, nc.gpsimd, nc.scalar
            sched = [
                (SY, kT, kT_d, 0, 512),
                (SC, kT, kT_d, 512, 1024),
                (GP, qT, qT_d, 0, 128),
                (GP, qT, qT_d, 128, 1024),
                (SC, wblk, wblk_d, 0, 512),
                (SY, qT, qT_d, 1024, 2048),
                (SC, wblk, wblk_d, 512, 1536),
                (GP, qT, qT_d, 2048, 3072),
                (SC, wblk, wblk_d, 1536, 2560),
                (SY, qT, qT_d, 3072, 4096),
                (GP, wblk, wblk_d, 2560, 3584),
                (SC, wblk, wblk_d, 3584, 4096),
                (SY, kT, kT_d, 1024, 2048),
                (GP, kT, kT_d, 2048, 3072),
                (SC, kT, kT_d, 3072, 4096),
                (SY, qT, qT_d, 4096, 5120),
                (GP, qT, qT_d, 5120, 6144),
                (SC, wblk, wblk_d, 4096, 5120),
                (SY, wblk, wblk_d, 5120, 6144),
                (GP, qT, qT_d, 6144, 7168),
                (SC, wblk, wblk_d, 6144, 7168),
                (SY, qT, qT_d, 7168, 8192),
                (GP, wblk, wblk_d, 7168, 8192),
            ]
            widths = {id(kT): n, id(qT): mh, id(wblk): wb_n}
            seen = set()
            for eng, dst, src, lo, hi in sched:
                hi = min(hi, widths[id(dst)])
                lo = min(lo, hi)
                key = (id(dst), lo, hi)
                if hi <= lo or key in seen:
                    continue
                seen.add(key)
                eng.dma_start(dst[:, lo:hi], src[:, lo:hi])
            # cover any remainder (larger-than-default configs)
            for dst, src, w in ((kT, kT_d, n), (qT, qT_d, mh), (wblk, wblk_d, wb_n)):
                done = max((h for (i_, l_, h) in seen if i_ == id(dst)), default=0)
                if done < w:
                    SY.dma_start(dst[:, done:], src[:, done:])

            # warm the ACT spline tables while DMAs run
            warm = const_pool.tile([128, 1], bf16)
            nc.gpsimd.memset(warm[:], 0)
            nc.scalar.activation(warm[:], warm[:],
                                 mybir.ActivationFunctionType.Relu)

            # warm the PE (HAM un-throttles after ~3.4us of activity) with
            # small matmuls on a zeroed scratch tile while DMAs run
            if n_tiles >= 16:
                wsrc = const_pool.tile([128, 128], bf16)
                nc.gpsimd.memset(wsrc[:], 0)
                wps = psum1.tile([128, 128], f32, tag="p1", name="warm_ps")
                for _ in range(45):
                    nc.tensor.matmul(wps[:], wsrc[:], wsrc[:],
                                     start=True, stop=True)

            def emit_mm1(g, hf, t):
                """mm1 for one mh-tile: c_per_half [128,512] psum tiles, each
                drained (relu+scale -> bf16) on a fixed engine per chunk."""
                tg = g * group_tiles + t
                qT_t = qT[:, bass.ts(tg, 128)]
                y_t = ypool.tile([128, n_chunk], bf16, tag="y")
                for c in range(c_per_half):
                    p1 = psum1.tile([128, 512], f32)
                    nc.tensor.matmul(
                        p1[:],
                        qT_t,
                        kT[:, bass.ds(hf * n_chunk + c * 512, 512)],
                        start=True,
                        stop=True,
                    )
                    ysl = y_t[:, bass.ts(c, 512)]
                    if (t * c_per_half + c) % 2 == 0:
                        nc.scalar.activation(
                            ysl, p1[:],
                            mybir.ActivationFunctionType.Relu,
                            scale=SOFTMAX_SCALE,
                        )
                    else:
                        nc.vector.tensor_scalar(
                            ysl, p1[:], SOFTMAX_SCALE, 0.0,
                            mybir.AluOpType.mult, mybir.AluOpType.max,
                        )
                return y_t

            def emit_mm2(p2_chunks, g, t, y_t):
                tg = g * group_tiles + t
                w_t = wblk[:, bass.ts(tg, gp)]
                for c in range(c_per_half):
                    nc.tensor.matmul(
                        p2_chunks[c][:],
                        w_t,
                        y_t[:, bass.ts(c, 512)],
                        start=(t == 0),
                        stop=(t == group_tiles - 1),
                    )

            DELAY = 2  # tiles of run-ahead before mm2 consumes a drained y
            for g in range(n_groups):
                for hf in range(n_halves):
                    p2_chunks = [psum2.tile([gp, 512], f32, tag="p2", name=f"p2_{g}_{hf}_{c}")
                                 for c in range(c_per_half)]
                    ys = []
                    for t in range(group_tiles):
                        ys.append(emit_mm1(g, hf, t))
                        if t >= DELAY:
                            emit_mm2(p2_chunks, g, t - DELAY, ys[t - DELAY])
                    for t in range(group_tiles - DELAY, group_tiles):
                        emit_mm2(p2_chunks, g, t, ys[t])
                    # per-chunk psum2 drain, alternating engines; stores
                    # split by partition halves across rotating queues so
                    # the final store isn't one long single-queue DMA
                    for c in range(c_per_half):
                        ost = ostage.tile([gp, 512], f32)
                        if (hf * c_per_half + c) % 2 == 0:
                            nc.vector.tensor_copy(ost[:], p2_chunks[c][:])
                        else:
                            nc.scalar.copy(ost[:], p2_chunks[c][:])
                        hp = gp // 2
                        qa = [nc.sync, nc.gpsimd, nc.scalar]
                        base = (g * n_halves + hf) * c_per_half + c
                        for piece in range(2):
                            rows = bass.ds(g * gp + piece * hp, hp)
                            nc.sync if False else None
                            qa[(base + piece) % 3].dma_start(
                                o_d[rows, bass.ds(hf * n_chunk + c * 512, 512)],
                                ost[bass.ds(piece * hp, hp), :],
                            )

    nc.compile()
    return nc


def marshal_core_inputs(q, k, weights, core, m_loc=M_LOC, group_tiles=32):
    """Host-side layout marshalling for one core (no arithmetic)."""
    n_tiles = (m_loc * H) // 128
    gp = 4 * group_tiles
    bf16 = ml_dtypes.bfloat16

    q_sh = np.asarray(q[0, core * m_loc:(core + 1) * m_loc])   # (m_loc, H, D) bf16
    qT = np.ascontiguousarray(q_sh.reshape(m_loc * H, D).T)     # (128, mh)
    kT = np.ascontiguousarray(np.asarray(k[0]).T)               # (128, n)

    w_sh = np.asarray(weights[core * m_loc:(core + 1) * m_loc, 0, :])  # (m_loc, H)
    # wblk[row, tg*gp + col]: for tile tg (4 m's), local m j (0..3), head h:
    #   row = 32*j + h, col = 4*(tg % group_tiles) + j  -> w[m, h]
    wblk = np.zeros((n_tiles, 128, gp), dtype=bf16)
    w_r = w_sh.reshape(n_tiles, 4, H)                           # (tg, j, h)
    tgs = np.arange(n_tiles)
    for j in range(4):
        cols = 4 * (tgs % group_tiles) + j                      # (tg,)
        wblk[tgs[:, None], 32 * j + np.arange(H)[None, :], cols[:, None]] = w_r[:, j, :]
    wblk = np.ascontiguousarray(wblk.transpose(1, 0, 2).reshape(128, n_tiles * gp))

    return {"qT": qT, "kT": kT, "wblk": wblk}


_NC_CACHE = {}


def _get_nc():
    if "nc" not in _NC_CACHE:
        _NC_CACHE["nc"] = build_nc()
    return _NC_CACHE["nc"]


def kernel(q, k, weights):
    nc = _get_nc()
    in_maps = [marshal_core_inputs(q, k, weights, c) for c in range(N_CORES)]
    res = run_bass_kernel_spmd(nc, in_maps, list(range(N_CORES)))
    out = np.concatenate([res.results[c]["o"] for c in range(N_CORES)], axis=0)
    return out[None]  # (1, M, N) fp32
